# revision 7
# baseline (speedup 1.0000x reference)
"""Trainium2 Bass kernel for nn_CumulativeIFFT.

Computes, for spectral (B=4, T=512, D=64, K=32, 2):
    s = spectral * sqrt(t+1)
    out[b,t,n,d] = (sum_k s_re[b,t,d,k]*cos(2pi n k/512)
                   - s_im[b,t,d,k]*sin(2pi n k/512)) / 512
Output: (4, 512, 512, 64) float32.

Formulation: per (b,t) pair, out[n,d] = sum_j WT[j,n] * X[j,d] where
j = 2k+ri flattens (k, re/im) and WT stacks [cos; -sin].

Host-side marshaling (not on the HW critical path):
  - x is pre-scaled by sqrt(t+1)/512, cast bf16, and TRANSPOSED to
    [J, TP, D] so the contraction axis lands on SBUF partitions with
    no on-device transpose at all.
  - the device writes out_dev[n, p, d] ([N, TP, D]); the host
    transposes back to [TP, N, D]. This layout makes every output DMA
    descriptor a contiguous 2-8KB run (full DMA bus rate) instead of
    the 512B runs a [TP, N, D] device layout would force.

Device program per core (TP=256 pairs):
  for r in 4 n-blocks of 128:            # wt[:, r] stays PE-stationary
    for chunk c in 32 (8 pairs each):
      psum[128, 8*64] = wt[:, rblk].T @ x[:, c]   # one matmul
      copy psum -> sbuf f16 (rotating Vector/Scalar/GpSimd engines)
    group up to 8 chunks -> one dma_start (128 descriptors x <=8KB)

Input loads are issued from four different sequencers in parallel with
ramped sizes so the first matmul can start as early as possible.

Sharding: 8 cores; core c handles b = c//2, t in [ (c%2)*256, ... ).
No cross-core communication.
"""

import math
import sys

import numpy as np

for _p in ("/opt/trn_rl_repo", "/root/.axon_site/_ro/trn_rl_repo"):
    if _p not in sys.path:
        sys.path.append(_p)

B, T, D, K = 4, 512, 64, 32
J = 2 * K          # flattened (k, re/im) contraction axis
N = 512            # output sequence length (seq_len)
NCORES = 8
TP = (B * T) // NCORES   # (b,t) pairs per core = 256
NB = N // 128            # 128-row output blocks = 4
NCHUNK = TP // 8         # 8-pair matmul chunks = 32

# input-load tiles (in pairs): ramped so chunk 0 is ready asap
XSIZES = [16, 48, 64, 128]
XBOUND = [0, 16, 64, 128, 256]

# chunks per output-DMA group: small head to start the store pipeline
# early, small tail to shorten the final drain.
GROUPS = {
    0: [1, 1, 2, 4, 8, 8, 8],
    1: [8, 8, 8, 8],
    2: [8, 8, 8, 8],
    3: [8, 8, 8, 4, 2, 2],
}

_CACHE = {}


def _build_program():
    import concourse.tile as tile
    from concourse import bacc, mybir

    f32 = mybir.dt.float32
    f16 = mybir.dt.float16
    bf16 = mybir.dt.bfloat16
    nc = bacc.Bacc("TRN2")

    x = nc.dram_tensor("x", [J, TP, D], bf16, kind="ExternalInput")
    wt = nc.dram_tensor("wt", [J, N], bf16, kind="ExternalInput")
    out = nc.dram_tensor("out", [N, TP, D], f16, kind="ExternalOutput")

    with tile.TileContext(nc) as tc:
        with (
            tc.tile_pool(name="const", bufs=1) as constp,
            tc.tile_pool(name="osb", bufs=3) as osbp,
            tc.tile_pool(name="ps", bufs=8, space="PSUM") as psp,
        ):
            wt_sb = constp.tile([J, N], bf16)
            nc.sync.dma_start(wt_sb[:], wt[:])
            xts = []
            # parallel issue: each input load on its own sequencer
            ldengines = [nc.scalar, nc.gpsimd, nc.sync, nc.scalar]
            for i, (p0, p1) in enumerate(zip(XBOUND[:-1], XBOUND[1:])):
                xt = constp.tile([J, p1 - p0, D], bf16, tag=f"xt{i}")
                ldengines[i % len(ldengines)].dma_start(xt[:], x[:, p0:p1, :])
                xts.append(xt)

            def xslice(c):
                p0 = c * 8
                for i, (lo, hi) in enumerate(zip(XBOUND[:-1], XBOUND[1:])):
                    if p0 >= lo and p0 < hi:
                        return xts[i][:, p0 - lo:p0 - lo + 8, :]
                raise AssertionError

            unit = 0
            for r in range(NB):
                c0 = 0
                for gsize in GROUPS[r]:
                    osb = osbp.tile([128, 64, D], f16, tag="osb")
                    for cc in range(gsize):
                        c = c0 + cc
                        ps = psp.tile([128, 8, D], f32, tag="ps")
                        nc.tensor.matmul(
                            ps[:],
                            wt_sb[:, r * 128:(r + 1) * 128],
                            xslice(c),
                            start=True,
                            stop=True,
                        )
                        dst = osb[:, cc * 8:(cc + 1) * 8, :]
                        if unit % 2 == 0:
                            nc.vector.tensor_copy(dst, ps[:])
                        else:
                            nc.scalar.copy(dst, ps[:])
                        unit += 1
                    nc.sync.dma_start(
                        out[r * 128:(r + 1) * 128, c0 * 8:(c0 + gsize) * 8, :],
                        osb[:, :gsize * 8, :],
                    )
                    c0 += gsize
    nc.compile()
    return nc


def _constants():
    n = np.arange(N, dtype=np.float32)
    k = np.arange(K, dtype=np.float32)
    ang = np.float32(2.0 * math.pi / N) * np.outer(n, k)  # (N, K) f32
    wt = np.empty((J, N), dtype=np.float32)
    wt[0::2, :] = np.cos(ang).T
    wt[1::2, :] = -np.sin(ang).T
    return wt


def _run(spectral: np.ndarray, trace: bool = False, **kw):
    from concourse import bass_utils
    import ml_dtypes

    bf16 = ml_dtypes.bfloat16

    spectral = np.ascontiguousarray(spectral, dtype=np.float32)
    assert spectral.shape == (B, T, D, K, 2)

    if "nc" not in _CACHE:
        _CACHE["nc"] = _build_program()
        _CACHE["wt"] = _constants().astype(bf16)
    nc = _CACHE["nc"]
    wt = _CACHE["wt"]

    thalf = T // 2
    in_maps = []
    for c in range(NCORES):
        b, t0 = c // 2, (c % 2) * thalf
        # fold the 1/N normalization into the per-position scale
        sc = np.sqrt(np.arange(t0 + 1, t0 + TP + 1, dtype=np.float32)) / N
        xc = (spectral[b, t0:t0 + thalf].reshape(TP, D, J)
              * sc[:, None, None]).astype(bf16)
        xc = np.ascontiguousarray(xc.transpose(2, 0, 1))  # [J, TP, D]
        in_maps.append({"x": xc, "wt": wt})

    res = bass_utils.run_bass_kernel_spmd(
        nc, in_maps, core_ids=list(range(NCORES)), trace=trace, **kw
    )

    out = np.empty((B, T, N, D), dtype=np.float32)
    for c in range(NCORES):
        b, t0 = c // 2, (c % 2) * thalf
        out[b, t0:t0 + thalf] = res.results[c]["out"].transpose(1, 0, 2)
    return out, res


def kernel(spectral: np.ndarray) -> np.ndarray:
    return _run(spectral, trace=False)[0]


# revision 8
# speedup vs baseline: 1.0151x; 1.0151x over previous
"""Trainium2 Bass kernel for nn_CumulativeIFFT.

Computes, for spectral (B=4, T=512, D=64, K=32, 2):
    s = spectral * sqrt(t+1)
    out[b,t,n,d] = (sum_k s_re[b,t,d,k]*cos(2pi n k/512)
                   - s_im[b,t,d,k]*sin(2pi n k/512)) / 512
Output: (4, 512, 512, 64) float32.

Formulation: per (b,t) pair, out[n,d] = sum_j WT[j,n] * X[j,d] where
j = 2k+ri flattens (k, re/im) and WT stacks [cos; -sin].

The GEMM runs in fp8 (e4m3) with MatmulPerfMode.DoubleRow, which
processes two contraction planes per partition at 0.5 cycles per
output row -- 2x the bf16 rate.  Plain fp8 would cost ~3% error, so
the contraction is error-compensated: with wh/wl = fp8 hi/lo split of
WT and xh/xl = fp8 hi/lo split of X, the 96-partition layout computes

    out = sum_j wh_j*(xh_j + xl_j)  (partitions 0-63, planes hi/lo)
        + sum_j wl_j*xh_j           (partitions 64-95, j-pairs)

dropping only the wl*xl term (~1e-3 relative, measured 1.2e-3 overall,
better than a bf16 GEMM).  X is scaled per source position by a power
of two s(t) ~ 2*512/sqrt(t+1) so fp8's narrow range is centered; the
host divides it back out during output reassembly.

Host-side marshaling (not on the HW critical path): X is pre-scaled,
split, and transposed to [96, 2, TP, D]; the device writes
out_dev[n, p, d] ([N, TP, D]) so every output-DMA descriptor is a
contiguous 2-8KB run; the host transposes back.

Device program per core (TP=256 pairs):
  for r in 4 n-blocks of 128:            # wt[:, r] stays PE-stationary
    for chunk c in 32 (8 pairs each):
      psum[128, 8*64] = DoubleRow-matmul(wt[:, rblk], x[:, c])
      copy psum -> sbuf f16 (alternating Vector/Scalar engines)
    group up to 8 chunks -> one dma_start (128 descriptors x <=8KB)

Sharding: 8 cores; core c handles b = c//2, t in [ (c%2)*256, ... ).
No cross-core communication.
"""

import math
import sys

import numpy as np

for _p in ("/opt/trn_rl_repo", "/root/.axon_site/_ro/trn_rl_repo"):
    if _p not in sys.path:
        sys.path.append(_p)

B, T, D, K = 4, 512, 64, 32
J = 2 * K          # flattened (k, re/im) contraction axis
JP = 96            # fp8 DoubleRow partitions: 64 wh-planes + 32 wl-pairs
N = 512            # output sequence length (seq_len)
NCORES = 8
TP = (B * T) // NCORES   # (b,t) pairs per core = 256
NB = N // 128            # 128-row output blocks = 4
NCHUNK = TP // 8         # 8-pair matmul chunks = 32

# input-load tiles (in pairs): ramped so chunk 0 is ready asap
XBOUND = [0, 16, 64, 128, 256]

# chunks per output-DMA group: small head to start the store pipeline
# early, small tail to shorten the final drain.
GROUPS = {
    0: [1, 1, 2, 4, 8, 8, 8],
    1: [8, 8, 8, 8],
    2: [8, 8, 8, 8],
    3: [8, 8, 8, 4, 2, 1, 1],
}

_CACHE = {}


def _build_program():
    import concourse.tile as tile
    from concourse import bacc, mybir

    f32 = mybir.dt.float32
    f16 = mybir.dt.float16
    fp8 = mybir.dt.float8e4
    nc = bacc.Bacc("TRN2")

    x = nc.dram_tensor("x", [JP, 2, TP, D], fp8, kind="ExternalInput")
    wt = nc.dram_tensor("wt", [JP, 2, N], fp8, kind="ExternalInput")
    out = nc.dram_tensor("out", [N, TP, D], f16, kind="ExternalOutput")

    with tile.TileContext(nc) as tc:
        with (
            tc.tile_pool(name="const", bufs=1) as constp,
            tc.tile_pool(name="osb", bufs=3) as osbp,
            tc.tile_pool(name="ps", bufs=8, space="PSUM") as psp,
        ):
            wt_sb = constp.tile([JP, 2, N], fp8)
            nc.sync.dma_start(wt_sb[:], wt[:])
            xts = []
            # parallel issue across the two HWDGE sequencers (gpsimd
            # SWDGE measured 11us+ for these loads -- avoid)
            ldengines = [nc.scalar, nc.sync, nc.scalar, nc.sync]
            for i, (p0, p1) in enumerate(zip(XBOUND[:-1], XBOUND[1:])):
                xt = constp.tile([JP, 2, p1 - p0, D], fp8, tag=f"xt{i}")
                ldengines[i].dma_start(xt[:], x[:, :, p0:p1, :])
                xts.append(xt)

            def xslice(c):
                p0 = c * 8
                for i, (lo, hi) in enumerate(zip(XBOUND[:-1], XBOUND[1:])):
                    if p0 >= lo and p0 < hi:
                        return xts[i][:, :, p0 - lo:p0 - lo + 8, :]
                raise AssertionError

            unit = 0
            for r in range(NB):
                c0 = 0
                for gsize in GROUPS[r]:
                    osb = osbp.tile([128, 64, D], f16, tag="osb")
                    for cc in range(gsize):
                        c = c0 + cc
                        ps = psp.tile([128, 8, D], f32, tag="ps")
                        nc.tensor.matmul(
                            ps[:],
                            wt_sb[:, :, r * 128:(r + 1) * 128],
                            xslice(c),
                            start=True,
                            stop=True,
                            perf_mode=mybir.MatmulPerfMode.DoubleRow,
                        )
                        dst = osb[:, cc * 8:(cc + 1) * 8, :]
                        if unit % 2 == 0:
                            nc.vector.tensor_copy(dst, ps[:])
                        else:
                            nc.scalar.copy(dst, ps[:])
                        unit += 1
                    nc.sync.dma_start(
                        out[r * 128:(r + 1) * 128, c0 * 8:(c0 + gsize) * 8, :],
                        osb[:, :gsize * 8, :],
                    )
                    c0 += gsize
    nc.compile()
    return nc


def _constants():
    import ml_dtypes

    fp8 = ml_dtypes.float8_e4m3

    n = np.arange(N, dtype=np.float32)
    k = np.arange(K, dtype=np.float32)
    ang = np.float32(2.0 * math.pi / N) * np.outer(n, k)  # (N, K) f32
    wt = np.empty((J, N), dtype=np.float32)
    wt[0::2, :] = np.cos(ang).T
    wt[1::2, :] = -np.sin(ang).T
    wh = wt.astype(fp8)
    wl = (wt - wh.astype(np.float32)).astype(fp8)
    wtp = np.empty((JP, 2, N), dtype=fp8)
    wtp[0:J, 0] = wh
    wtp[0:J, 1] = wh
    wtp[J:JP, 0] = wl[0::2]
    wtp[J:JP, 1] = wl[1::2]
    return wtp


def _run(spectral: np.ndarray, trace: bool = False, **kw):
    from concourse import bass_utils
    import ml_dtypes

    fp8 = ml_dtypes.float8_e4m3

    spectral = np.ascontiguousarray(spectral, dtype=np.float32)
    assert spectral.shape == (B, T, D, K, 2)

    if "nc" not in _CACHE:
        _CACHE["nc"] = _build_program()
        _CACHE["wt"] = _constants()
    nc = _CACHE["nc"]
    wtp = _CACHE["wt"]

    thalf = T // 2
    in_maps = []
    inv_s = []
    for c in range(NCORES):
        b, t0 = c // 2, (c % 2) * thalf
        # fold the 1/N normalization into the per-position scale, then
        # center fp8's range with a per-position power-of-two
        sc = np.sqrt(np.arange(t0 + 1, t0 + TP + 1, dtype=np.float32)) / N
        s = np.exp2(np.round(np.log2(2.0 / sc))).astype(np.float32)
        xp = (spectral[b, t0:t0 + thalf].reshape(TP, D, J)
              * (sc * s)[:, None, None]).transpose(2, 0, 1)  # [J, TP, D]
        xh = xp.astype(fp8)
        xl = (xp - xh.astype(np.float32)).astype(fp8)
        xin = np.empty((JP, 2, TP, D), dtype=fp8)
        xin[0:J, 0] = xh
        xin[0:J, 1] = xl
        xin[J:JP, 0] = xh[0::2]
        xin[J:JP, 1] = xh[1::2]
        in_maps.append({"x": xin, "wt": wtp})
        inv_s.append((1.0 / s).astype(np.float32))

    res = bass_utils.run_bass_kernel_spmd(
        nc, in_maps, core_ids=list(range(NCORES)), trace=trace, **kw
    )

    out = np.empty((B, T, N, D), dtype=np.float32)
    for c in range(NCORES):
        b, t0 = c // 2, (c % 2) * thalf
        out[b, t0:t0 + thalf] = (
            res.results[c]["out"].transpose(1, 0, 2)
            * inv_s[c][:, None, None]
        )
    return out, res


def kernel(spectral: np.ndarray) -> np.ndarray:
    return _run(spectral, trace=False)[0]
